# revision 31
# baseline (speedup 1.0000x reference)
"""CrossAttentionBlock kernel for Trainium2 (8 NeuronCores, SPMD data-parallel).

Problem (hardcoded from spec):
  B=2, N=M=2048, D=1024, H=8 heads, DH=32 (multi-query: single shared K/V head),
  FF=4096, eps=1e-5. gamma is folded into the weights host-side.

Sharding: pure data-parallel over the 4096 (batch, token) rows of x.
  Core c handles 512 query tokens: batch b = c // 4, rows 512*(c%4) .. +512.
  Each core computes K/V for its full batch (2048 keys), attention + SwiGLU FFN
  for its 512 tokens. No collectives; host concatenates the 8 [512, 1024]
  outputs.

v4 design notes (baseline 557us, v2 389us, v3 438us):
  * The scalar engine pays ~1.28us (ACT_TABLE_LOAD) every time the activation
    FUNCTION changes. The kernel is therefore phase-pure: A/B use only Sqrt,
    the FF1 phase only Silu, the attention phase only Exp - 3 table loads total
    (v2/v3 interleaved FF1 with attention and thrashed Exp<->Silu every
    iteration, which dominated phase D).
  * sim (Q.K) runs as 4 concurrent row-tiled K=32 matmuls (tile_position=(32a,0)),
    one head per 32-row group, against a 4x-replicated K. The replication is free:
    the K projection's stationary holds [w_k|w_k|w_k|w_k].
  * LayerNorm is never applied to y. K/V are projected from RAW y and the fold
    correction a_k (x) (-mean) is a K=1 matmul accumulated into the projection's
    own PSUM group; the rstd scaling is a single DVE multiply on the way out.
  * V carries 32 ones-columns, so P@V yields the softmax denominator already
    broadcast to 32 partitions: finalize is reciprocal + multiply on DVE only.
    Two heads share each PSUM bank via col-group placement (tile_position (0,0)
    and (0,64), M=64).
  * Q packed 4 heads per matmul; attention out-projection and FFN down-projection
    accumulate into the SAME PSUM banks (single accumulation group), so the final
    output needs no adds.
  * All big matmuls are bf16 (halves w1/w2 DMA); LN statistics stay f32r on raw
    activations. Softmax runs without max subtraction (|sim| < ~7 for N(0,1)
    data).
"""
import sys

if "/opt/trn_rl_repo" not in sys.path:
    sys.path.insert(0, "/opt/trn_rl_repo")

import numpy as np
import ml_dtypes

import concourse.bass as bass
import concourse.bacc as bacc
import concourse.mybir as mybir
import concourse.tile as tile
import time as _time
_T0 = _time.time()
def _tick(msg):
    print(f"[{_time.time()-_T0:7.1f}s] {msg}", flush=True)
from concourse.bass_utils import run_bass_kernel_spmd

F32 = mybir.dt.float32
F32R = mybir.dt.float32r
BF16 = mybir.dt.bfloat16

B, N, M, D = 2, 2048, 2048, 1024
H, DH = 8, 32
FF = 4 * D
EPS = 1e-5
R = 512            # tokens per core
NCORES = 8
SCALE = DH ** -0.5
BF = ml_dtypes.bfloat16

AF = mybir.ActivationFunctionType
ALU = mybir.AluOpType


def build_nc():
    nc = bacc.Bacc("TRN2", target_bir_lowering=False, debug=False,
                   num_devices=NCORES)

    # ---- DRAM I/O (per-core views, host-prepared layouts) ----
    # feature-major activations: [ki, ko, token] with feature = ko*128 + ki
    xT = nc.dram_tensor("xT", [128, 8, R], F32R, kind="ExternalInput")
    yT = nc.dram_tensor("yT", [128, 8, M], F32R, kind="ExternalInput")
    # wq4: [ki, ko, j, 32a+e] = SCALE * w_q[ko*128+ki, (4j+a)*32+e]
    wq4 = nc.dram_tensor("wq4", [128, 8, 2, 128], BF16, kind="ExternalInput")
    # wkv4: cols 0:128 = w_k replicated 4x, cols 128:160 = w_v
    wkv4 = nc.dram_tensor("wkv4", [128, 8, 160], F32R, kind="ExternalInput")
    # akv: cols 0:128 = column sums of w_k replicated 4x, 128:160 = w_v col sums
    akv = nc.dram_tensor("akv", [1, 160], F32R, kind="ExternalInput")
    # wout4: [32a+f, j, d] = w_out[(4j+a)*32+f, d]
    wout4 = nc.dram_tensor("wout4", [128, 2, D], BF16, kind="ExternalInput")
    # w_ff1 val/gate-paired: [pair, ki, ko, 256] (cols 0:128 val, 128:256 gate)
    w1 = nc.dram_tensor("w1", [32, 128, 8, 256], BF16, kind="ExternalInput")
    # w_ff2: [ki, ko, d] with ff_feature = ko*128 + ki
    w2 = nc.dram_tensor("w2", [128, 32, D], BF16, kind="ExternalInput")
    ident = nc.dram_tensor("ident", [128, 128], BF16, kind="ExternalInput")
    out = nc.dram_tensor("out", [R, D], F32, kind="ExternalOutput")
    out_r = out.rearrange("(mo ki) d -> ki mo d", ki=128)

    with tile.TileContext(nc) as tc:
        persist_scope = tc.tile_pool(name="persist", bufs=1)
        persist = persist_scope.__enter__()

        # ---- constants ----
        ones_t = persist.tile([128, 128], F32R)
        ident_t = persist.tile([128, 128], BF16)
        nc.sync.dma_start(ident_t[:], ident[:])
        ones_f32 = persist.tile([128, 128], F32)
        nc.vector.memset(ones_f32[:], 1.0)
        nc.vector.tensor_copy(ones_t[:], ones_f32[:])
        eps_t = persist.tile([128, 1], F32)
        nc.vector.memset(eps_t[:], EPS)
        akv_t = persist.tile([1, 160], F32R)
        nc.sync.dma_start(akv_t[:], akv[:])

        # ---- persistent activations ----
        xnB = persist.tile([128, 8, R], BF16)        # LN(x), bf16
        hT = persist.tile([128, 32, R], BF16)        # SwiGLU hidden
        kT_rep = persist.tile([128, 16, 128], BF16)  # K, 4x replicated per chunk
        # V token-major; cols 32:64 stay all-ones -> denominator rows in P@V
        v_aug = persist.tile([128, 16, 2 * DH], BF16)
        qpack = persist.tile([128, 2, R], BF16)      # Q packed 4 heads per group
        attn_out4 = persist.tile([128, 2, R], BF16)  # rescaled attn, head-major

        nc.vector.memset(v_aug[:], 1.0)

        def ln_stats(src_t, scratch, psln):
            """negmean/rstd (broadcast to 128 partitions) of a raw feature-major
            [128, 8, R] tile. Stats via all-ones stationary matmuls."""
            sq = scratch.tile([128, 8, R], F32R, tag="ln_sq", bufs=2)
            nc.vector.tensor_mul(sq[:], src_t[:], src_t[:])
            s_ps = psln.tile([128, R], F32, tag="ln_s", bufs=2)
            ss_ps = psln.tile([128, R], F32, tag="ln_ss", bufs=2)
            for ko in range(8):
                nc.tensor.matmul(s_ps[:], ones_t[:], src_t[:, ko, :],
                                 start=(ko == 0), stop=(ko == 7))
            for ko in range(8):
                nc.tensor.matmul(ss_ps[:], ones_t[:], sq[:, ko, :],
                                 start=(ko == 0), stop=(ko == 7))
            negmean = scratch.tile([128, R], F32R, tag="ln_nm", bufs=2)
            nc.vector.tensor_scalar_mul(negmean[:], s_ps[:], -1.0 / D)
            msq = scratch.tile([128, R], F32, tag="ln_msq", bufs=2)
            nc.vector.tensor_mul(msq[:], negmean[:], negmean[:])
            var = scratch.tile([128, R], F32, tag="ln_var", bufs=2)
            nc.vector.scalar_tensor_tensor(
                var[:], ss_ps[:], 1.0 / D, msq[:], ALU.mult, ALU.subtract)
            sd = scratch.tile([128, R], F32, tag="ln_sd", bufs=2)
            nc.scalar.activation(sd[:], var[:], AF.Sqrt, bias=eps_t[:])
            rstd = scratch.tile([128, R], F32, tag="ln_rstd", bufs=2)
            nc.vector.reciprocal(rstd[:], sd[:])
            return negmean, rstd

        _tick("Phase A+B")
        # ====== Phase A+B: LN(x) and K/V from raw y with LN fold ======
        ffA_scope = tc.tile_pool(name="ffA", bufs=3)
        ffA = ffA_scope.__enter__()
        phA_scope = tc.tile_pool(name="phA", bufs=1)
        phA = phA_scope.__enter__()
        psLN_scope = tc.tile_pool(name="psLN", bufs=1, space="PSUM")
        psLN = psLN_scope.__enter__()
        psB_scope = tc.tile_pool(name="psB", bufs=1, space="PSUM")
        psB = psB_scope.__enter__()

        wkv_t = phA.tile([128, 8, 160], F32R, tag="wkv")
        nc.sync.dma_start(wkv_t[:], wkv4[:])

        xt = phA.tile([128, 8, R], F32R, tag="xt")
        nc.sync.dma_start(xt[:], xT[:])
        negmean_x, rstd_x = ln_stats(xt, phA, psLN)

        # y group 0 stats go out before the x apply so the PE isn't
        # head-of-line blocked behind DVE work.
        yts = [phA.tile([128, 8, R], F32R, tag="yt", bufs=2, name=f"yt{i}")
               for i in range(2)]
        nc.sync.dma_start(yts[0][:], yT[:, :, 0:R])
        negmean_y, rstd_y = ln_stats(yts[0], phA, psLN)
        nc.sync.dma_start(yts[1][:], yT[:, :, R:2 * R])

        # prefetch the first FF1 weight pairs during B
        w1_pre = [ffA.tile([128, 8, 256], BF16, tag="w1", bufs=3,
                           name=f"w1p{i}") for i in range(3)]
        for i in range(3):
            nc.sync.dma_start(w1_pre[i][:], w1[i])

        # snapshot x stats: the ln_* tag slots get reused by the y groups
        c2x = phA.tile([128, R], F32, tag="c2x")
        nc.vector.tensor_mul(c2x[:], negmean_x[:], rstd_x[:])
        rstdx = phA.tile([128, R], F32, tag="rstdx")
        nc.vector.tensor_copy(rstdx[:], rstd_x[:])

        def apply_x(kos):
            for ko in kos:
                tmp = phA.tile([128, R], F32, tag="ln_tmp", bufs=2)
                nc.vector.tensor_mul(tmp[:], xt[:, ko, :], rstdx[:])
                nc.vector.tensor_add(xnB[:, ko, :], tmp[:], c2x[:])

        for g in range(4):
            yt = yts[g % 2]
            # raw projections + LN-fold correction (K=1 matmul) in one group
            k4_ps = psB.tile([128, R], F32, tag="k4")
            v_ps = psB.tile([DH, R], F32, tag="v")
            for ko in range(8):
                nc.tensor.matmul(k4_ps[:], wkv_t[:, ko, 0:128], yt[:, ko, :],
                                 start=(ko == 0), stop=False)
            nc.tensor.matmul(k4_ps[:], akv_t[:, 0:128], negmean_y[0:1, :],
                             start=False, stop=True)
            for ko in range(8):
                nc.tensor.matmul(v_ps[:], wkv_t[:, ko, 128:160], yt[:, ko, :],
                                 start=(ko == 0), stop=False)
            nc.tensor.matmul(v_ps[:], akv_t[:, 128:160], negmean_y[0:1, :],
                             start=False, stop=True)
            if g < 3:  # next group's stats overlap this group's epilogue
                nxt = yts[(g + 1) % 2]
                negmean_n, rstd_n = ln_stats(nxt, phA, psLN)
            if g < 2:  # g+2's DMA into the slot this group just finished with
                nc.sync.dma_start(yts[g % 2][:],
                                  yT[:, :, (g + 2) * R:(g + 3) * R])
            # scale by rstd on the way out
            nc.vector.tensor_mul(kT_rep[:, 4 * g:4 * g + 4, :], k4_ps[:],
                                 rstd_y[:])
            vstage = phA.tile([DH, R], BF16, tag="vstage", bufs=2)
            nc.vector.tensor_mul(vstage[:], v_ps[:], rstd_y[0:DH, :])
            if g < 3:
                negmean_y, rstd_y = negmean_n, rstd_n
            # transpose V chunks into v_aug (token-major)
            for c in range(4):
                kc = 4 * g + c
                tr_ps = psB.tile([128, DH], BF16, tag="tr", bufs=2)
                nc.tensor.transpose(tr_ps[:], vstage[:, c * 128:(c + 1) * 128],
                                    ident_t[:DH, :DH])
                nc.vector.tensor_copy(v_aug[:, kc, 0:DH], tr_ps[:])
            if g < 2:  # spread the x apply so it doesn't block y chains
                apply_x(range(4 * g, 4 * g + 4))

        _tick("Phase C")
        # ================= Phase C: Q proj (packed 4 heads) =================
        wq_t = phA.tile([128, 8, 2, 128], BF16, tag="wq")
        nc.sync.dma_start(wq_t[:], wq4[:])
        for j in range(2):
            q_ps = psB.tile([128, R], F32, tag="k4")  # reuse K's PSUM slot
            for ko in range(8):
                nc.tensor.matmul(q_ps[:], wq_t[:, ko, j, :], xnB[:, ko, :],
                                 start=(ko == 0), stop=(ko == 7))
            nc.vector.tensor_copy(qpack[:, j, :], q_ps[:])

        psB_scope.__exit__(None, None, None)
        psLN_scope.__exit__(None, None, None)
        phA_scope.__exit__(None, None, None)

        # prefetch phase E+G weights well ahead of use
        phG_scope = tc.tile_pool(name="phG", bufs=1)
        phG = phG_scope.__enter__()
        wout_t = phG.tile([128, 2, D], BF16, tag="wout")
        nc.sync.dma_start(wout_t[:], wout4[:])
        w2_ts = [phG.tile([128, 8, D], BF16, tag="w2", bufs=2, name=f"w2t{i}")
                 for i in range(2)]
        nc.sync.dma_start(w2_ts[0][:], w2[:, 0:8, :])
        nc.sync.dma_start(w2_ts[1][:], w2[:, 8:16, :])

        _tick("Phase F")
        # ================= Phase F: FF1 (dense, Silu only) =================
        psFF_scope = tc.tile_pool(name="psFF", bufs=1, space="PSUM")
        psFF = psFF_scope.__enter__()

        for pair in range(32):
            if pair < 3:
                w1_t = w1_pre[pair]
            else:
                w1_t = ffA.tile([128, 8, 256], BF16, tag="w1", bufs=3)
                nc.sync.dma_start(w1_t[:], w1[pair])
            val_ps = psFF.tile([128, R], F32, tag="ffv", bufs=2)
            gate_ps = psFF.tile([128, R], F32, tag="ffg", bufs=2)
            for ko in range(8):
                nc.tensor.matmul(val_ps[:], w1_t[:, ko, 0:128],
                                 xnB[:, ko, :], start=(ko == 0), stop=(ko == 7))
            for ko in range(8):
                nc.tensor.matmul(gate_ps[:], w1_t[:, ko, 128:256],
                                 xnB[:, ko, :], start=(ko == 0), stop=(ko == 7))
            sg = ffA.tile([128, R], BF16, tag="sg", bufs=2)
            nc.scalar.activation(sg[:], gate_ps[:], AF.Silu)
            nc.vector.tensor_mul(hT[:, pair, :], val_ps[:], sg[:])

        psFF_scope.__exit__(None, None, None)

        _tick("Phase D")
        # ================= Phase D: attention (Exp only) =================
        phD_scope = tc.tile_pool(name="phD", bufs=1)
        phD = phD_scope.__enter__()
        psD_scope = tc.tile_pool(name="psD", bufs=1, space="PSUM")
        psD = psD_scope.__enter__()

        for j in range(2):
            av = [psD.tile([128, R], F32, tag=f"av{u}", name=f"av{j}{u}",
                           bufs=2)
                  for u in range(2)]
            for kc in range(16):
                # two 2-bank sim tiles, double-buffered via the shared tag, so
                # the next iteration's sim matmuls overlap this one's EXP
                simA = psD.tile([128, 2, R], F32, tag="sim", bufs=2,
                                name="simA")
                simB = psD.tile([128, 2, R], F32, tag="sim", bufs=2,
                                name="simB")
                for a in range(4):
                    dst = simA if a < 2 else simB
                    nc.tensor.matmul(dst[:, a % 2, :],
                                     kT_rep[32 * a:32 * a + 32, kc, :],
                                     qpack[32 * a:32 * a + 32, j, :],
                                     start=True, stop=True,
                                     tile_position=(32 * a, 0))
                p_t = phD.tile([128, 4, R], BF16, tag="p", bufs=2)
                nc.scalar.activation(p_t[:, 0:2, :], simA[:], AF.Exp)
                nc.scalar.activation(p_t[:, 2:4, :], simB[:], AF.Exp)
                for a in range(4):
                    p0 = 64 * (a % 2)
                    nc.tensor.matmul(av[a // 2][p0:p0 + 2 * DH, :],
                                     v_aug[:, kc, :], p_t[:, a, :],
                                     start=(kc == 0), stop=(kc == 15),
                                     tile_position=(0, p0))
            # finalize: denominator rows come out of P@V already broadcast to
            # 32 partitions; batch all 4 heads into one reciprocal (its DVE
            # cost is flat in partition count)
            denall = phD.tile([128, R], F32, tag="denall", bufs=2)
            for a in range(4):
                p0 = 64 * (a % 2)
                nc.vector.tensor_copy(denall[32 * a:32 * a + 32, :],
                                      av[a // 2][p0 + DH:p0 + 2 * DH, :])
            rall = phD.tile([128, R], F32, tag="rall", bufs=2)
            nc.vector.reciprocal(rall[:], denall[:])
            for a in range(4):
                p0 = 64 * (a % 2)
                nc.vector.tensor_mul(attn_out4[32 * a:32 * a + 32, j, :],
                                     av[a // 2][p0:p0 + DH, :],
                                     rall[32 * a:32 * a + 32, :])

        psD_scope.__exit__(None, None, None)
        phD_scope.__exit__(None, None, None)

        _tick("Phase E+G")
        # ========== Phase E+G: out-proj and FF2 into shared PSUM ==========
        psG_scope = tc.tile_pool(name="psG", bufs=1, space="PSUM")
        psG = psG_scope.__enter__()

        f2 = [psG.tile([128, 2, R], F32, tag=f"f2_{mo}", name=f"f2_{mo}")
              for mo in range(4)]
        # attention out-projection opens the accumulation groups
        for mo in range(4):
            mo_sl = slice(mo * 128, (mo + 1) * 128)
            for nh in range(2):
                for j in range(2):
                    nc.tensor.matmul(f2[mo][:, nh, :],
                                     attn_out4[:, j, mo_sl],
                                     wout_t[:, j, nh * R:(nh + 1) * R],
                                     start=(j == 0), stop=False)
        # FF2 accumulates on top
        for blk in range(4):
            if blk < 2:
                w2_t = w2_ts[blk]
            else:
                w2_t = phG.tile([128, 8, D], BF16, tag="w2", bufs=2)
                nc.sync.dma_start(w2_t[:], w2[:, blk * 8:(blk + 1) * 8, :])
            for kf in range(8):
                kfg = blk * 8 + kf
                for mo in range(4):
                    mo_sl = slice(mo * 128, (mo + 1) * 128)
                    for nh in range(2):
                        nc.tensor.matmul(
                            f2[mo][:, nh, :],
                            hT[:, kfg, mo_sl],
                            w2_t[:, kf, nh * R:(nh + 1) * R],
                            start=False, stop=(kfg == 31))
        for mo in range(4):
            out_t = phG.tile([128, D], F32, tag="out_t", bufs=2)
            nc.vector.tensor_copy(out_t[:], f2[mo][:])
            nc.sync.dma_start(out_r[:, mo, :], out_t[:])

        psG_scope.__exit__(None, None, None)
        phG_scope.__exit__(None, None, None)
        ffA_scope.__exit__(None, None, None)
        persist_scope.__exit__(None, None, None)

    _tick("tile scheduling done, bacc compile")
    nc.compile()
    _tick("bacc compile done")
    return nc


def _prep_inputs(x, y, gamma, w_q, w_kv, w_out, w_ff1, w_ff2):
    """Host-side relayout. Returns (shared_map, per_core_xT, per_batch_yT)."""
    f32 = np.float32
    g = np.asarray(gamma, f32)
    # fold LayerNorm weight into the consumers of the normed activations
    w_q = np.asarray(w_q, f32) * g[:, None]
    w_kv = np.asarray(w_kv, f32) * g[:, None]
    w_ff1 = np.asarray(w_ff1, f32) * g[:, None]

    def fm(a, ko, dt=f32):  # [K, F] -> [128, ko, F] feature-major grouping
        K, F_ = a.shape
        return np.ascontiguousarray(
            a.reshape(ko, 128, F_).transpose(1, 0, 2)).astype(dt)

    wq_s = (w_q * SCALE).reshape(8, 128, 2, 4 * DH)   # [ko, ki, j, ae]
    wq4 = np.ascontiguousarray(wq_s.transpose(1, 0, 2, 3)).astype(BF)

    wk, wv = w_kv[:, :DH], w_kv[:, DH:]
    wkv4 = np.concatenate([np.tile(wk, (1, 4)), wv], axis=1)  # [D, 160]
    akv = np.concatenate([np.tile(wk.sum(axis=0), 4),
                          wv.sum(axis=0)]).reshape(1, 160).astype(f32)

    # wout4[32a+f, j, :] = w_out[(4j+a)*32+f, :]
    wout4 = np.empty((128, 2, D), f32)
    w_out = np.asarray(w_out, f32)
    for j in range(2):
        for a in range(4):
            wout4[32 * a:32 * a + 32, j, :] = \
                w_out[(4 * j + a) * DH:(4 * j + a + 1) * DH, :]
    wout4 = wout4.astype(BF)

    w1p = np.empty((32, 128, 8, 256), dtype=BF)
    for i in range(32):
        blk = np.concatenate(
            [w_ff1[:, i * 128:(i + 1) * 128],
             w_ff1[:, FF + i * 128:FF + (i + 1) * 128]], axis=1)  # [1024, 256]
        w1p[i] = blk.reshape(8, 128, 256).transpose(1, 0, 2).astype(BF)

    shared = {
        "wq4": wq4,
        "wkv4": fm(wkv4, 8),
        "akv": akv,
        "wout4": wout4,
        "w1": w1p,
        "w2": fm(np.asarray(w_ff2, f32), 32, BF),
        "ident": np.eye(128, dtype=f32).astype(BF),
    }

    xTs = []
    for c in range(NCORES):
        b, r0 = c // 4, (c % 4) * R
        xc = np.ascontiguousarray(x[b, r0:r0 + R, :].T)      # [1024, 512]
        xTs.append(fm(xc, 8))
    yTs = [fm(np.ascontiguousarray(y[b].T), 8) for b in range(B)]
    return shared, xTs, yTs


_NC_CACHE = None


def _get_nc():
    global _NC_CACHE
    if _NC_CACHE is None:
        _NC_CACHE = build_nc()
    return _NC_CACHE


def run(x, y, w_q, w_kv, w_out, w_ff1, w_ff2, gamma=None, **spmd_kwargs):
    if gamma is None:
        gamma = np.ones((D,), np.float32)
    shared, xTs, yTs = _prep_inputs(x, y, gamma, w_q, w_kv, w_out, w_ff1, w_ff2)
    in_maps = [dict(shared, xT=xTs[c], yT=yTs[c // 4]) for c in range(NCORES)]
    nc = _get_nc()
    res = run_bass_kernel_spmd(nc, in_maps, core_ids=list(range(NCORES)),
                               **spmd_kwargs)
    outs = [r["out"] for r in res.results]
    full = np.concatenate(outs, axis=0).reshape(B, N, D).astype(np.float32)
    return full, res


def kernel(x, y, gamma, w_q, w_kv, w_out, w_ff1, w_ff2):
    x = np.asarray(x, dtype=np.float32)
    y = np.asarray(y, dtype=np.float32)
    full, _ = run(np.asarray(x), np.asarray(y), np.asarray(w_q),
                  np.asarray(w_kv), np.asarray(w_out), np.asarray(w_ff1),
                  np.asarray(w_ff2), gamma=np.asarray(gamma))
    return full


# revision 34
# speedup vs baseline: 1.2104x; 1.2104x over previous
"""CrossAttentionBlock kernel for Trainium2 (8 NeuronCores, SPMD data-parallel).

Problem (hardcoded from spec):
  B=2, N=M=2048, D=1024, H=8 heads, DH=32 (multi-query: single shared K/V head),
  FF=4096, eps=1e-5. gamma is folded into the weights host-side.

Sharding: pure data-parallel over the 4096 (batch, token) rows of x.
  Core c handles 512 query tokens: batch b = c // 4, rows 512*(c%4) .. +512.
  Each core computes K/V for its full batch (2048 keys), attention + SwiGLU FFN
  for its 512 tokens. No collectives; host concatenates the 8 [512, 1024]
  outputs.

v4 design notes (baseline 557us, v2 389us, v3 438us):
  * The scalar engine pays ~1.28us (ACT_TABLE_LOAD) every time the activation
    FUNCTION changes. The kernel is therefore phase-pure: A/B use only Sqrt,
    the FF1 phase only Silu, the attention phase only Exp - 3 table loads total
    (v2/v3 interleaved FF1 with attention and thrashed Exp<->Silu every
    iteration, which dominated phase D).
  * sim (Q.K) runs as 4 concurrent row-tiled K=32 matmuls (tile_position=(32a,0)),
    one head per 32-row group, against a 4x-replicated K. The replication is free:
    the K projection's stationary holds [w_k|w_k|w_k|w_k].
  * LayerNorm is never applied to y. K/V are projected from RAW y and the fold
    correction a_k (x) (-mean) is a K=1 matmul accumulated into the projection's
    own PSUM group; the rstd scaling is a single DVE multiply on the way out.
  * V carries 32 ones-columns, so P@V yields the softmax denominator already
    broadcast to 32 partitions: finalize is reciprocal + multiply on DVE only.
    Two heads share each PSUM bank via col-group placement (tile_position (0,0)
    and (0,64), M=64).
  * Q packed 4 heads per matmul; attention out-projection and FFN down-projection
    accumulate into the SAME PSUM banks (single accumulation group), so the final
    output needs no adds.
  * All big matmuls are bf16 (halves w1/w2 DMA); LN statistics stay f32r on raw
    activations. Softmax runs without max subtraction (|sim| < ~7 for N(0,1)
    data).
"""
import sys

if "/opt/trn_rl_repo" not in sys.path:
    sys.path.insert(0, "/opt/trn_rl_repo")

import numpy as np
import ml_dtypes

import concourse.bass as bass
import concourse.bacc as bacc
import concourse.mybir as mybir
import concourse.tile as tile
import time as _time
_T0 = _time.time()
def _tick(msg):
    print(f"[{_time.time()-_T0:7.1f}s] {msg}", flush=True)
from concourse.bass_utils import run_bass_kernel_spmd

F32 = mybir.dt.float32
F32R = mybir.dt.float32r
BF16 = mybir.dt.bfloat16

B, N, M, D = 2, 2048, 2048, 1024
H, DH = 8, 32
FF = 4 * D
EPS = 1e-5
R = 512            # tokens per core
NCORES = 8
SCALE = DH ** -0.5
BF = ml_dtypes.bfloat16

AF = mybir.ActivationFunctionType
ALU = mybir.AluOpType


def build_nc():
    nc = bacc.Bacc("TRN2", target_bir_lowering=False, debug=False,
                   num_devices=NCORES)

    # ---- DRAM I/O (per-core views, host-prepared layouts) ----
    # feature-major activations: [ki, ko, token] with feature = ko*128 + ki
    xT = nc.dram_tensor("xT", [128, 8, R], F32R, kind="ExternalInput")
    yT = nc.dram_tensor("yT", [128, 8, M], F32R, kind="ExternalInput")
    # wq4: [ki, ko, j, 32a+e] = SCALE * w_q[ko*128+ki, (4j+a)*32+e]
    wq4 = nc.dram_tensor("wq4", [128, 8, 2, 128], BF16, kind="ExternalInput")
    # wkv4: cols 0:128 = w_k replicated 4x, cols 128:160 = w_v
    wkv4 = nc.dram_tensor("wkv4", [128, 8, 160], F32R, kind="ExternalInput")
    # akv: cols 0:128 = column sums of w_k replicated 4x, 128:160 = w_v col sums
    akv = nc.dram_tensor("akv", [1, 160], F32R, kind="ExternalInput")
    # wout4: [32a+f, j, d] = w_out[(4j+a)*32+f, d]
    wout4 = nc.dram_tensor("wout4", [128, 2, D], BF16, kind="ExternalInput")
    # w_ff1 val/gate-paired: [pair, ki, ko, 256] (cols 0:128 val, 128:256 gate)
    w1 = nc.dram_tensor("w1", [32, 128, 8, 256], BF16, kind="ExternalInput")
    # w_ff2: [ki, ko, d] with ff_feature = ko*128 + ki
    w2 = nc.dram_tensor("w2", [128, 32, D], BF16, kind="ExternalInput")
    ident = nc.dram_tensor("ident", [128, 128], BF16, kind="ExternalInput")
    out = nc.dram_tensor("out", [R, D], F32, kind="ExternalOutput")
    out_r = out.rearrange("(mo ki) d -> ki mo d", ki=128)

    with tile.TileContext(nc) as tc:
        persist_scope = tc.tile_pool(name="persist", bufs=1)
        persist = persist_scope.__enter__()

        # ---- constants ----
        ones_t = persist.tile([128, 128], F32R)
        ident_t = persist.tile([128, 128], BF16)
        nc.sync.dma_start(ident_t[:], ident[:])
        ones_f32 = persist.tile([128, 128], F32)
        nc.vector.memset(ones_f32[:], 1.0)
        nc.vector.tensor_copy(ones_t[:], ones_f32[:])
        eps_t = persist.tile([128, 1], F32)
        nc.vector.memset(eps_t[:], EPS)
        akv_t = persist.tile([1, 160], F32R)
        nc.sync.dma_start(akv_t[:], akv[:])

        # ---- persistent activations ----
        xnB = persist.tile([128, 8, R], BF16)        # LN(x), bf16
        hT = persist.tile([128, 32, R], BF16)        # SwiGLU hidden
        kT_rep = persist.tile([128, 16, 128], BF16)  # K, 4x replicated per chunk
        # V token-major; cols 32:64 stay all-ones -> denominator rows in P@V
        v_aug = persist.tile([128, 16, 2 * DH], BF16)
        qpack = persist.tile([128, 2, R], BF16)      # Q packed 4 heads per group
        attn_out4 = persist.tile([128, 2, R], BF16)  # rescaled attn, head-major

        nc.vector.memset(v_aug[:], 1.0)

        def ln_stats(src_t, scratch, psln):
            """negmean/rstd (broadcast to 128 partitions) of a raw feature-major
            [128, 8, R] tile. Stats via all-ones stationary matmuls."""
            sq = scratch.tile([128, 8, R], F32R, tag="ln_sq", bufs=2)
            nc.vector.tensor_mul(sq[:], src_t[:], src_t[:])
            s_ps = psln.tile([128, R], F32, tag="ln_s", bufs=2)
            ss_ps = psln.tile([128, R], F32, tag="ln_ss", bufs=2)
            for ko in range(8):
                nc.tensor.matmul(s_ps[:], ones_t[:], src_t[:, ko, :],
                                 start=(ko == 0), stop=(ko == 7))
            for ko in range(8):
                nc.tensor.matmul(ss_ps[:], ones_t[:], sq[:, ko, :],
                                 start=(ko == 0), stop=(ko == 7))
            negmean = scratch.tile([128, R], F32R, tag="ln_nm", bufs=2)
            nc.vector.tensor_scalar_mul(negmean[:], s_ps[:], -1.0 / D)
            msq = scratch.tile([128, R], F32, tag="ln_msq", bufs=2)
            nc.vector.tensor_mul(msq[:], negmean[:], negmean[:])
            var = scratch.tile([128, R], F32, tag="ln_var", bufs=2)
            nc.vector.scalar_tensor_tensor(
                var[:], ss_ps[:], 1.0 / D, msq[:], ALU.mult, ALU.subtract)
            sd = scratch.tile([128, R], F32, tag="ln_sd", bufs=2)
            nc.scalar.activation(sd[:], var[:], AF.Sqrt, bias=eps_t[:])
            rstd = scratch.tile([128, R], F32, tag="ln_rstd", bufs=2)
            nc.vector.reciprocal(rstd[:], sd[:])
            return negmean, rstd

        _tick("Phase A+B")
        # ====== Phase A+B: LN(x) and K/V from raw y with LN fold ======
        ffA_scope = tc.tile_pool(name="ffA", bufs=3)
        ffA = ffA_scope.__enter__()
        phA_scope = tc.tile_pool(name="phA", bufs=1)
        phA = phA_scope.__enter__()
        psLN_scope = tc.tile_pool(name="psLN", bufs=1, space="PSUM")
        psLN = psLN_scope.__enter__()
        psB_scope = tc.tile_pool(name="psB", bufs=1, space="PSUM")
        psB = psB_scope.__enter__()

        wkv_t = phA.tile([128, 8, 160], F32R, tag="wkv")
        nc.sync.dma_start(wkv_t[:], wkv4[:])

        xt = phA.tile([128, 8, R], F32R, tag="xt")
        nc.sync.dma_start(xt[:], xT[:])
        negmean_x, rstd_x = ln_stats(xt, phA, psLN)

        # y group 0 stats go out before the x apply so the PE isn't
        # head-of-line blocked behind DVE work.
        yts = [phA.tile([128, 8, R], F32R, tag="yt", bufs=2, name=f"yt{i}")
               for i in range(2)]
        nc.sync.dma_start(yts[0][:], yT[:, :, 0:R])
        negmean_y, rstd_y = ln_stats(yts[0], phA, psLN)
        nc.sync.dma_start(yts[1][:], yT[:, :, R:2 * R])

        # prefetch the first FF1 weight pairs during B
        w1_pre = [ffA.tile([128, 8, 256], BF16, tag="w1", bufs=3,
                           name=f"w1p{i}") for i in range(3)]
        for i in range(3):
            nc.sync.dma_start(w1_pre[i][:], w1[i])

        # snapshot x stats: the ln_* tag slots get reused by the y groups
        c2x = phA.tile([128, R], F32, tag="c2x")
        nc.vector.tensor_mul(c2x[:], negmean_x[:], rstd_x[:])
        rstdx = phA.tile([128, R], F32, tag="rstdx")
        nc.vector.tensor_copy(rstdx[:], rstd_x[:])

        def apply_x(kos):
            for ko in kos:
                tmp = phA.tile([128, R], F32, tag="ln_tmp", bufs=2)
                nc.vector.tensor_mul(tmp[:], xt[:, ko, :], rstdx[:])
                nc.vector.tensor_add(xnB[:, ko, :], tmp[:], c2x[:])

        for g in range(4):
            yt = yts[g % 2]
            # raw projections + LN-fold correction (K=1 matmul) in one group
            k4_ps = psB.tile([128, R], F32, tag="k4")
            v_ps = psB.tile([DH, R], F32, tag="v")
            for ko in range(8):
                nc.tensor.matmul(k4_ps[:], wkv_t[:, ko, 0:128], yt[:, ko, :],
                                 start=(ko == 0), stop=False)
            nc.tensor.matmul(k4_ps[:], akv_t[:, 0:128], negmean_y[0:1, :],
                             start=False, stop=True)
            for ko in range(8):
                nc.tensor.matmul(v_ps[:], wkv_t[:, ko, 128:160], yt[:, ko, :],
                                 start=(ko == 0), stop=False)
            nc.tensor.matmul(v_ps[:], akv_t[:, 128:160], negmean_y[0:1, :],
                             start=False, stop=True)
            if g < 3:  # next group's stats overlap this group's epilogue
                nxt = yts[(g + 1) % 2]
                negmean_n, rstd_n = ln_stats(nxt, phA, psLN)
            if g < 2:  # g+2's DMA into the slot this group just finished with
                nc.sync.dma_start(yts[g % 2][:],
                                  yT[:, :, (g + 2) * R:(g + 3) * R])
            # scale by rstd on the way out
            nc.vector.tensor_mul(kT_rep[:, 4 * g:4 * g + 4, :], k4_ps[:],
                                 rstd_y[:])
            vstage = phA.tile([DH, R], BF16, tag="vstage", bufs=2)
            nc.vector.tensor_mul(vstage[:], v_ps[:], rstd_y[0:DH, :])
            if g < 3:
                negmean_y, rstd_y = negmean_n, rstd_n
            # transpose V chunks into v_aug (token-major)
            for c in range(4):
                kc = 4 * g + c
                tr_ps = psB.tile([128, DH], BF16, tag="tr", bufs=2)
                nc.tensor.transpose(tr_ps[:], vstage[:, c * 128:(c + 1) * 128],
                                    ident_t[:DH, :DH])
                nc.vector.tensor_copy(v_aug[:, kc, 0:DH], tr_ps[:])
            if g < 2:  # spread the x apply so it doesn't block y chains
                apply_x(range(4 * g, 4 * g + 4))

        _tick("Phase C")
        # ================= Phase C: Q proj (packed 4 heads) =================
        wq_t = phA.tile([128, 8, 2, 128], BF16, tag="wq")
        nc.sync.dma_start(wq_t[:], wq4[:])
        for j in range(2):
            q_ps = psB.tile([128, R], F32, tag="k4")  # reuse K's PSUM slot
            for ko in range(8):
                nc.tensor.matmul(q_ps[:], wq_t[:, ko, j, :], xnB[:, ko, :],
                                 start=(ko == 0), stop=(ko == 7))
            nc.vector.tensor_copy(qpack[:, j, :], q_ps[:])

        psB_scope.__exit__(None, None, None)
        psLN_scope.__exit__(None, None, None)
        phA_scope.__exit__(None, None, None)

        # prefetch phase E+G weights well ahead of use
        phG_scope = tc.tile_pool(name="phG", bufs=1)
        phG = phG_scope.__enter__()
        wout_t = phG.tile([128, 2, D], BF16, tag="wout")
        nc.sync.dma_start(wout_t[:], wout4[:])
        w2_ts = [phG.tile([128, 8, D], BF16, tag="w2", bufs=2, name=f"w2t{i}")
                 for i in range(2)]
        nc.sync.dma_start(w2_ts[0][:], w2[:, 0:8, :])
        nc.sync.dma_start(w2_ts[1][:], w2[:, 8:16, :])

        _tick("Phase F")
        # ================= Phase F: FF1 (dense, Silu only) =================
        psFF_scope = tc.tile_pool(name="psFF", bufs=1, space="PSUM")
        psFF = psFF_scope.__enter__()

        for pair in range(32):
            if pair < 3:
                w1_t = w1_pre[pair]
            else:
                w1_t = ffA.tile([128, 8, 256], BF16, tag="w1", bufs=3)
                nc.sync.dma_start(w1_t[:], w1[pair])
            val_ps = psFF.tile([128, R], F32, tag="ffv", bufs=2)
            gate_ps = psFF.tile([128, R], F32, tag="ffg", bufs=2)
            for ko in range(8):
                nc.tensor.matmul(val_ps[:], w1_t[:, ko, 0:128],
                                 xnB[:, ko, :], start=(ko == 0), stop=(ko == 7))
            for ko in range(8):
                nc.tensor.matmul(gate_ps[:], w1_t[:, ko, 128:256],
                                 xnB[:, ko, :], start=(ko == 0), stop=(ko == 7))
            sg = ffA.tile([128, R], BF16, tag="sg", bufs=2)
            nc.scalar.activation(sg[:], gate_ps[:], AF.Silu)
            nc.vector.tensor_mul(hT[:, pair, :], val_ps[:], sg[:])

        psFF_scope.__exit__(None, None, None)

        _tick("Phase D")
        # ================= Phase D: attention (Exp only) =================
        phD_scope = tc.tile_pool(name="phD", bufs=1)
        phD = phD_scope.__enter__()
        psD_scope = tc.tile_pool(name="psD", bufs=1, space="PSUM")
        psD = psD_scope.__enter__()

        for j in range(2):
            av = [psD.tile([128, R], F32, tag=f"av{u}", name=f"av{j}{u}",
                           bufs=1)
                  for u in range(2)]
            for kc in range(16):
                # two 2-bank sim tiles, double-buffered via the shared tag, so
                # the next iteration's sim matmuls overlap this one's EXP
                simA = psD.tile([128, 2, R], F32, tag="sim", bufs=3,
                                name="simA")
                simB = psD.tile([128, 2, R], F32, tag="sim", bufs=3,
                                name="simB")
                for a in range(4):
                    dst = simA if a < 2 else simB
                    nc.tensor.matmul(dst[:, a % 2, :],
                                     kT_rep[32 * a:32 * a + 32, kc, :],
                                     qpack[32 * a:32 * a + 32, j, :],
                                     start=True, stop=True,
                                     tile_position=(32 * a, 0))
                p_t = phD.tile([128, 4, R], BF16, tag="p", bufs=2)
                nc.scalar.activation(p_t[:, 0:2, :], simA[:], AF.Exp)
                nc.scalar.activation(p_t[:, 2:4, :], simB[:], AF.Exp)
                for a in range(4):
                    p0 = 64 * (a % 2)
                    nc.tensor.matmul(av[a // 2][p0:p0 + 2 * DH, :],
                                     v_aug[:, kc, :], p_t[:, a, :],
                                     start=(kc == 0), stop=(kc == 15),
                                     tile_position=(0, p0))
            # finalize: denominator rows come out of P@V already broadcast to
            # 32 partitions; batch all 4 heads into one reciprocal (its DVE
            # cost is flat in partition count)
            denall = phD.tile([128, R], F32, tag="denall", bufs=2)
            for a in range(4):
                p0 = 64 * (a % 2)
                nc.vector.tensor_copy(denall[32 * a:32 * a + 32, :],
                                      av[a // 2][p0 + DH:p0 + 2 * DH, :])
            rall = phD.tile([128, R], F32, tag="rall", bufs=2)
            nc.vector.reciprocal(rall[:], denall[:])
            for a in range(4):
                p0 = 64 * (a % 2)
                nc.vector.tensor_mul(attn_out4[32 * a:32 * a + 32, j, :],
                                     av[a // 2][p0:p0 + DH, :],
                                     rall[32 * a:32 * a + 32, :])

        psD_scope.__exit__(None, None, None)
        phD_scope.__exit__(None, None, None)

        _tick("Phase E+G")
        # ========== Phase E+G: out-proj and FF2 into shared PSUM ==========
        psG_scope = tc.tile_pool(name="psG", bufs=1, space="PSUM")
        psG = psG_scope.__enter__()

        f2 = [psG.tile([128, 2, R], F32, tag=f"f2_{mo}", name=f"f2_{mo}")
              for mo in range(4)]
        # FF2 opens the accumulation groups (hT/w2 are ready the moment the
        # PE frees up; the out-projection would wait on the j=1 finalize)
        for blk in range(4):
            if blk < 2:
                w2_t = w2_ts[blk]
            else:
                w2_t = phG.tile([128, 8, D], BF16, tag="w2", bufs=2)
                nc.sync.dma_start(w2_t[:], w2[:, blk * 8:(blk + 1) * 8, :])
            for kf in range(8):
                kfg = blk * 8 + kf
                for mo in range(4):
                    mo_sl = slice(mo * 128, (mo + 1) * 128)
                    for nh in range(2):
                        nc.tensor.matmul(
                            f2[mo][:, nh, :],
                            hT[:, kfg, mo_sl],
                            w2_t[:, kf, nh * R:(nh + 1) * R],
                            start=(kfg == 0), stop=False)
        # attention out-projection closes the accumulation groups
        for mo in range(4):
            mo_sl = slice(mo * 128, (mo + 1) * 128)
            for nh in range(2):
                for j in range(2):
                    nc.tensor.matmul(f2[mo][:, nh, :],
                                     attn_out4[:, j, mo_sl],
                                     wout_t[:, j, nh * R:(nh + 1) * R],
                                     start=False, stop=(j == 1))
        for mo in range(4):
            out_t = phG.tile([128, D], F32, tag="out_t", bufs=2)
            nc.vector.tensor_copy(out_t[:], f2[mo][:])
            nc.sync.dma_start(out_r[:, mo, :], out_t[:])

        psG_scope.__exit__(None, None, None)
        phG_scope.__exit__(None, None, None)
        ffA_scope.__exit__(None, None, None)
        persist_scope.__exit__(None, None, None)

    _tick("tile scheduling done, bacc compile")
    nc.compile()
    _tick("bacc compile done")
    return nc


def _prep_inputs(x, y, gamma, w_q, w_kv, w_out, w_ff1, w_ff2):
    """Host-side relayout. Returns (shared_map, per_core_xT, per_batch_yT)."""
    f32 = np.float32
    g = np.asarray(gamma, f32)
    # fold LayerNorm weight into the consumers of the normed activations
    w_q = np.asarray(w_q, f32) * g[:, None]
    w_kv = np.asarray(w_kv, f32) * g[:, None]
    w_ff1 = np.asarray(w_ff1, f32) * g[:, None]

    def fm(a, ko, dt=f32):  # [K, F] -> [128, ko, F] feature-major grouping
        K, F_ = a.shape
        return np.ascontiguousarray(
            a.reshape(ko, 128, F_).transpose(1, 0, 2)).astype(dt)

    wq_s = (w_q * SCALE).reshape(8, 128, 2, 4 * DH)   # [ko, ki, j, ae]
    wq4 = np.ascontiguousarray(wq_s.transpose(1, 0, 2, 3)).astype(BF)

    wk, wv = w_kv[:, :DH], w_kv[:, DH:]
    wkv4 = np.concatenate([np.tile(wk, (1, 4)), wv], axis=1)  # [D, 160]
    akv = np.concatenate([np.tile(wk.sum(axis=0), 4),
                          wv.sum(axis=0)]).reshape(1, 160).astype(f32)

    # wout4[32a+f, j, :] = w_out[(4j+a)*32+f, :]
    wout4 = np.empty((128, 2, D), f32)
    w_out = np.asarray(w_out, f32)
    for j in range(2):
        for a in range(4):
            wout4[32 * a:32 * a + 32, j, :] = \
                w_out[(4 * j + a) * DH:(4 * j + a + 1) * DH, :]
    wout4 = wout4.astype(BF)

    w1p = np.empty((32, 128, 8, 256), dtype=BF)
    for i in range(32):
        blk = np.concatenate(
            [w_ff1[:, i * 128:(i + 1) * 128],
             w_ff1[:, FF + i * 128:FF + (i + 1) * 128]], axis=1)  # [1024, 256]
        w1p[i] = blk.reshape(8, 128, 256).transpose(1, 0, 2).astype(BF)

    shared = {
        "wq4": wq4,
        "wkv4": fm(wkv4, 8),
        "akv": akv,
        "wout4": wout4,
        "w1": w1p,
        "w2": fm(np.asarray(w_ff2, f32), 32, BF),
        "ident": np.eye(128, dtype=f32).astype(BF),
    }

    xTs = []
    for c in range(NCORES):
        b, r0 = c // 4, (c % 4) * R
        xc = np.ascontiguousarray(x[b, r0:r0 + R, :].T)      # [1024, 512]
        xTs.append(fm(xc, 8))
    yTs = [fm(np.ascontiguousarray(y[b].T), 8) for b in range(B)]
    return shared, xTs, yTs


_NC_CACHE = None


def _get_nc():
    global _NC_CACHE
    if _NC_CACHE is None:
        _NC_CACHE = build_nc()
    return _NC_CACHE


def run(x, y, w_q, w_kv, w_out, w_ff1, w_ff2, gamma=None, **spmd_kwargs):
    if gamma is None:
        gamma = np.ones((D,), np.float32)
    shared, xTs, yTs = _prep_inputs(x, y, gamma, w_q, w_kv, w_out, w_ff1, w_ff2)
    in_maps = [dict(shared, xT=xTs[c], yT=yTs[c // 4]) for c in range(NCORES)]
    nc = _get_nc()
    res = run_bass_kernel_spmd(nc, in_maps, core_ids=list(range(NCORES)),
                               **spmd_kwargs)
    outs = [r["out"] for r in res.results]
    full = np.concatenate(outs, axis=0).reshape(B, N, D).astype(np.float32)
    return full, res


def kernel(x, y, gamma, w_q, w_kv, w_out, w_ff1, w_ff2):
    x = np.asarray(x, dtype=np.float32)
    y = np.asarray(y, dtype=np.float32)
    full, _ = run(np.asarray(x), np.asarray(y), np.asarray(w_q),
                  np.asarray(w_kv), np.asarray(w_out), np.asarray(w_ff1),
                  np.asarray(w_ff2), gamma=np.asarray(gamma))
    return full
